# revision 1
# baseline (speedup 1.0000x reference)
"""TRN2 Bass kernel for GNN message passing (segment-sum of gathered node rows).

out[r] = sum over edges e with row[e]==r of x[col[e]]   (N=100000, E=2000000, D=32)

Strategy (8 NeuronCores, SPMD):
  - Edges sharded by target row: core c owns rows [c*12500, (c+1)*12500).
    Outputs are disjoint per core -> no all-reduce.
  - Per core, edges split into 4 col-buckets of 25000 cols so gather indices
    fit int16 for the gpsimd dma_gather custom op (256B elems: x rows padded
    from 128B to 256B).
  - Per (core, bucket), nodes are packed into "windows" of <=31 node slots
    and exactly 128 edge slots (one PE chunk). Per chunk: one-hot S[128,32]
    built on DVE via is_equal(li, iota); one fp32 matmul S^T @ msgs
    accumulates the 32 window rows into a PSUM band (window%4); groups of 4
    windows share a [128,32] PSUM tile, copied to SBUF staging; one DMA out.
  - Host packs windows, builds the int16 index image (wrapped in 16
    partitions, replicated to all 8 Q7 stripes), and un-permutes the output.

The gather is bound by SWDGE Q7 descriptor generation (~9ns/index measured),
so pad edges are minimized by equal-width buckets + greedy degree packing.
"""

import numpy as np

import concourse.bass as bass
import concourse.bacc as bacc
import concourse.mybir as mybir
import concourse.tile as tile
from concourse.bass_utils import run_bass_kernel_spmd

N_NODES = 100000
N_EDGES = 2000000
D = 32
EPAD = 64            # padded row length (f32) -> 256B dma_gather elems
NC = 8
ROWS_PER_CORE = N_NODES // NC
BUCKET = 25000
NBUCKETS = 4
WIN_EDGES = 128
TRASH = 31
MAX_REAL = 31
CALL_CHUNKS = 64
SBATCH = 16


def _pack_windows(degrees):
    n = len(degrees)
    order = np.argsort(degrees, kind="stable")[::-1]
    win_id = np.empty(n, np.int32)
    slot = np.empty(n, np.int32)
    degs = degrees[order]
    if n and degs[0] > WIN_EDGES:
        raise ValueError("node degree exceeds window capacity")
    hi, lo = 0, n - 1
    w = 0
    while hi <= lo:
        cap = WIN_EDGES
        s = 0
        while hi <= lo and s < MAX_REAL and degs[hi] <= cap:
            win_id[order[hi]] = w
            slot[order[hi]] = s
            cap -= degs[hi]
            s += 1
            hi += 1
        while hi <= lo and s < MAX_REAL and degs[lo] <= cap:
            win_id[order[lo]] = w
            slot[order[lo]] = s
            cap -= degs[lo]
            s += 1
            lo -= 1
        w += 1
    return win_id, slot, w


def _preprocess(x, edge_index):
    x = np.ascontiguousarray(np.asarray(x, dtype=np.float32))
    ei = np.asarray(edge_index)
    row = ei[0].astype(np.int64)
    col = ei[1].astype(np.int64)

    x_pad = np.zeros((N_NODES, EPAD), np.float32)
    x_pad[:, :D] = x
    iota = np.tile(np.arange(D, dtype=np.float32), (128, 1))

    core = (row // ROWS_PER_CORE).astype(np.int64)
    bucket = (col // BUCKET).astype(np.int64)

    key = (core * NBUCKETS + bucket) * N_NODES + row
    key_order = np.argsort(key, kind="stable")
    srow = row[key_order]
    scol = col[key_order]
    skey_cb = (core * NBUCKETS + bucket)[key_order]

    cb_info = []
    for c in range(NC):
        for b in range(NBUCKETS):
            m0 = np.searchsorted(skey_cb, c * NBUCKETS + b)
            m1 = np.searchsorted(skey_cb, c * NBUCKETS + b + 1)
            rows_cb = srow[m0:m1]
            cols_cb = scol[m0:m1]
            nod, start, deg = np.unique(rows_cb, return_index=True, return_counts=True)
            win_id, slot, nwin = _pack_windows(deg)
            cb_info.append(
                dict(nod=nod, start=start, deg=deg, cols=cols_cb,
                     win_id=win_id, slot=slot, nwin=nwin)
            )

    NW = []
    for b in range(NBUCKETS):
        mx = max(cb_info[c * NBUCKETS + b]["nwin"] for c in range(NC))
        NW.append(((mx + 3) // 4) * 4)
    NW_total = sum(NW)
    n_sw = NW_total // 4
    wg_of_b = np.concatenate([[0], np.cumsum(NW)])[:NBUCKETS]

    in_maps = []
    metas = []
    for c in range(NC):
        col16 = np.zeros((NW_total, WIN_EDGES), np.int16)
        li = np.full((NW_total, WIN_EDGES), TRASH, np.float32)
        meta_nid = []
        meta_w = []
        meta_slot = []
        for b in range(NBUCKETS):
            info = cb_info[c * NBUCKETS + b]
            nod, start, deg = info["nod"], info["start"], info["deg"]
            cols_cb, win_id, slot, nwin = (
                info["cols"], info["win_id"], info["slot"], info["nwin"]
            )
            if len(nod) == 0:
                continue
            ordn = np.lexsort((slot, win_id))
            w_sorted = win_id[ordn]
            d_sorted = deg[ordn]
            cum = np.cumsum(d_sorted) - d_sorted
            first_idx = np.searchsorted(w_sorted, np.arange(nwin))
            off_in_win = cum - cum[first_idx][w_sorted]
            off_node = np.empty(len(nod), np.int64)
            off_node[ordn] = off_in_win
            g0 = wg_of_b[b]
            t_node = (g0 + win_id.astype(np.int64)) * WIN_EDGES + off_node
            within = np.arange(len(cols_cb)) - np.repeat(start, deg)
            targets = np.repeat(t_node, deg) + within
            col16.reshape(-1)[targets] = (cols_cb - b * BUCKET).astype(np.int16)
            li.reshape(-1)[targets] = np.repeat(slot, deg).astype(np.float32)
            meta_nid.append(nod)
            meta_w.append(g0 + win_id.astype(np.int64))
            meta_slot.append(slot.astype(np.int64))

        li_img = li.T.copy()
        idx_img = np.zeros((128, NW_total * 8), np.int16)
        for b in range(NBUCKETS):
            g0 = wg_of_b[b]
            for call0 in range(0, NW[b], CALL_CHUNKS):
                nch = min(CALL_CHUNKS, NW[b] - call0)
                lin = col16[g0 + call0 : g0 + call0 + nch].reshape(-1)
                ncols = nch * 8
                ii = np.arange(nch * WIN_EDGES)
                img16 = np.zeros((16, ncols), np.int16)
                img16[ii % 16, ii // 16] = lin
                cbase = (g0 + call0) * 8
                for k in range(8):
                    idx_img[16 * k : 16 * (k + 1), cbase : cbase + ncols] = img16

        in_maps.append(
            {"xpad": x_pad, "idx_img": idx_img, "li_img": li_img, "iota": iota}
        )
        metas.append(
            (
                np.concatenate(meta_nid) if meta_nid else np.zeros(0, np.int64),
                np.concatenate(meta_w) if meta_w else np.zeros(0, np.int64),
                np.concatenate(meta_slot) if meta_slot else np.zeros(0, np.int64),
            )
        )

    meta = {"NW": NW, "NW_total": NW_total, "n_sw": n_sw, "metas": metas}
    return in_maps, meta


def _assemble_output(results, meta):
    out = np.zeros((N_NODES, D), np.float32)
    n_sw = meta["n_sw"]
    for c in range(NC):
        st = np.asarray(results[c]["out"]).reshape(128, n_sw, D)
        nid, w, slot = meta["metas"][c]
        vals = st[(w % 4) * 32 + slot, w // 4, :]
        np.add.at(out, nid, vals)
    return out


def _build_program(NW, with_reps=False):
    NW_total = sum(NW)
    n_sw = NW_total // 4
    nc = bacc.Bacc(
        "TRN2",
        target_bir_lowering=False,
        debug=False,
        enable_asserts=False,
        num_devices=NC,
    )
    dt = mybir.dt
    xpad_d = nc.dram_tensor("xpad", [N_NODES, EPAD], dt.float32, kind="ExternalInput")
    idx_img_d = nc.dram_tensor("idx_img", [128, NW_total * 8], dt.int16, kind="ExternalInput")
    li_img_d = nc.dram_tensor("li_img", [128, NW_total], dt.float32, kind="ExternalInput")
    iota_d = nc.dram_tensor("iota", [128, D], dt.float32, kind="ExternalInput")
    if with_reps:
        reps_d = nc.dram_tensor("reps", [1, 2], dt.int32, kind="ExternalInput")
    out_d = nc.dram_tensor("out", [128, n_sw * D], dt.float32, kind="ExternalOutput")

    with tile.TileContext(nc) as tc:
        with (
            tc.tile_pool(name="const", bufs=1) as cp,
            tc.tile_pool(name="msgs", bufs=3) as mp,
            tc.tile_pool(name="sbat", bufs=3) as sbp,
            tc.tile_pool(name="psum", bufs=4, space="PSUM") as pp,
        ):
            idx_img = cp.tile([128, NW_total * 8], dt.int16)
            li_img = cp.tile([128, NW_total], dt.float32)
            iota_t = cp.tile([128, D], dt.float32)
            nc.sync.dma_start(out=idx_img[:], in_=idx_img_d.ap())
            nc.sync.dma_start(out=li_img[:], in_=li_img_d.ap())
            nc.sync.dma_start(out=iota_t[:], in_=iota_d.ap())
            staging = cp.tile([128, n_sw * D], dt.float32)

            def body():
                w_abs = 0
                s16 = None
                ps = None
                for b in range(NBUCKETS):
                    lo = b * BUCKET
                    hi = min((b + 1) * BUCKET, N_NODES)
                    xs = xpad_d.ap()[lo:hi, :]
                    for call0 in range(0, NW[b], CALL_CHUNKS):
                        nch = min(CALL_CHUNKS, NW[b] - call0)
                        w0 = w_abs + call0
                        mg = mp.tile([128, CALL_CHUNKS, EPAD], dt.float32, tag="mg")
                        nc.gpsimd.dma_gather(
                            out_ap=mg[:, 0:nch, :],
                            in_ap=xs,
                            idxs_ap=idx_img[:, w0 * 8 : (w0 + nch) * 8],
                            num_idxs=nch * WIN_EDGES,
                            num_idxs_reg=nch * WIN_EDGES,
                            elem_size=EPAD,
                            single_packet=False,
                        )
                        for k in range(nch):
                            w = w0 + k
                            if w % SBATCH == 0:
                                nb = min(SBATCH, NW_total - w)
                                s16 = sbp.tile([128, SBATCH, D], dt.float32, tag="s16")
                                li_slice = li_img[:, w : w + nb]
                                nc.vector.tensor_tensor(
                                    out=s16[:, 0:nb, :],
                                    in0=bass.AP(
                                        li_img.tensor,
                                        li_slice.offset,
                                        [li_slice.ap[0], [1, nb], [0, D]],
                                    ),
                                    in1=bass.AP(
                                        iota_t.tensor,
                                        iota_t[:].offset,
                                        [iota_t[:].ap[0], [0, nb], [1, D]],
                                    ),
                                    op=mybir.AluOpType.is_equal,
                                )
                            if w % 4 == 0:
                                ps = pp.tile([128, D], dt.float32, space="PSUM", tag="ps")
                            nc.tensor.matmul(
                                out=ps[32 * (w % 4) : 32 * (w % 4) + 32, :],
                                lhsT=s16[:, w % SBATCH, 0:32],
                                rhs=mg[:, k, 0:D],
                                start=True,
                                stop=True,
                                tile_position=(0, 32 * (w % 4)),
                            )
                            if w % 4 == 3:
                                sw = w // 4
                                nc.scalar.copy(staging[:, sw * D : (sw + 1) * D], ps[:])
                    w_abs += NW[b]
                nc.sync.dma_start(out=out_d.ap(), in_=staging[:])

            if with_reps:
                reps_t = cp.tile([1, 2], dt.int32)
                nc.sync.dma_start(out=reps_t[:], in_=reps_d.ap())
                r = nc.values_load(reps_t[0:1, 0:1])
                with tc.For_i(0, r):
                    body()
            else:
                body()

    nc.compile()
    return nc


_program_cache = {}


def kernel(x, edge_index):
    in_maps, meta = _preprocess(x, edge_index)
    key = tuple(meta["NW"])
    if key not in _program_cache:
        _program_cache[key] = _build_program(meta["NW"], with_reps=False)
    nc = _program_cache[key]
    res = run_bass_kernel_spmd(nc, in_maps, core_ids=list(range(NC)))
    return _assemble_output(res.results, meta)



# revision 5
# speedup vs baseline: 14.2354x; 14.2354x over previous
"""TRN2 Bass kernel for GNN message passing (segment-sum of gathered node rows).

out[r] = sum over edges e with row[e]==r of x[col[e]]   (N=100000, E=2000000, D=32)

Strategy (8 NeuronCores, SPMD):
  - Edges sharded by target row: core c owns rows [c*12500, (c+1)*12500).
    Outputs are disjoint per core -> no all-reduce.
  - Per core, edges split into 4 col-buckets of 25000 cols so gather indices
    fit int16 for the gpsimd dma_gather custom op (256B elems: x rows padded
    from 128B to 256B).
  - Per (core, bucket), nodes are packed into "windows" of <=31 node slots
    and exactly 128 edge slots (one PE chunk). Per chunk: one-hot S[128,32]
    built on DVE via is_equal(li, iota); one fp32 matmul S^T @ msgs
    accumulates the 32 window rows into a PSUM band (window%4); groups of 4
    windows share a [128,32] PSUM tile, copied to SBUF staging; one DMA out.
  - Host packs windows, builds the int16 index image (wrapped in 16
    partitions, replicated to all 8 Q7 stripes), and un-permutes the output.

The gather is bound by SWDGE Q7 descriptor generation (~9ns/index measured),
so pad edges are minimized by equal-width buckets + greedy degree packing.
"""

import numpy as np

import concourse.bass as bass
import concourse.bacc as bacc
import concourse.mybir as mybir
import concourse.tile as tile
from concourse.bass_utils import run_bass_kernel_spmd

N_NODES = 100000
N_EDGES = 2000000
D = 32
EPAD = 64            # padded row length (f32) -> 256B dma_gather elems
NC = 8
ROWS_PER_CORE = N_NODES // NC
BUCKET = 25000
NBUCKETS = 4
WIN_EDGES = 128
TRASH = 31
MAX_REAL = 31
CALL_CHUNKS = 32
SBATCH = 16
NSWQ = 4


def _pack_windows(degrees):
    n = len(degrees)
    order = np.argsort(degrees, kind="stable")[::-1]
    win_id = np.empty(n, np.int32)
    slot = np.empty(n, np.int32)
    degs = degrees[order]
    if n and degs[0] > WIN_EDGES:
        raise ValueError("node degree exceeds window capacity")
    hi, lo = 0, n - 1
    w = 0
    while hi <= lo:
        cap = WIN_EDGES
        s = 0
        while hi <= lo and s < MAX_REAL and degs[hi] <= cap:
            win_id[order[hi]] = w
            slot[order[hi]] = s
            cap -= degs[hi]
            s += 1
            hi += 1
        while hi <= lo and s < MAX_REAL and degs[lo] <= cap:
            win_id[order[lo]] = w
            slot[order[lo]] = s
            cap -= degs[lo]
            s += 1
            lo -= 1
        w += 1
    return win_id, slot, w


def _preprocess(x, edge_index):
    x = np.ascontiguousarray(np.asarray(x, dtype=np.float32))
    ei = np.asarray(edge_index)
    row = ei[0].astype(np.int64)
    col = ei[1].astype(np.int64)

    x_pad = np.zeros((N_NODES, EPAD), np.float32)
    x_pad[:, :D] = x
    iota = np.tile(np.arange(D, dtype=np.float32), (128, 1))

    core = (row // ROWS_PER_CORE).astype(np.int64)
    bucket = (col // BUCKET).astype(np.int64)

    key = (core * NBUCKETS + bucket) * N_NODES + row
    key_order = np.argsort(key, kind="stable")
    srow = row[key_order]
    scol = col[key_order]
    skey_cb = (core * NBUCKETS + bucket)[key_order]

    cb_info = []
    for c in range(NC):
        for b in range(NBUCKETS):
            m0 = np.searchsorted(skey_cb, c * NBUCKETS + b)
            m1 = np.searchsorted(skey_cb, c * NBUCKETS + b + 1)
            rows_cb = srow[m0:m1]
            cols_cb = scol[m0:m1]
            nod, start, deg = np.unique(rows_cb, return_index=True, return_counts=True)
            win_id, slot, nwin = _pack_windows(deg)
            cb_info.append(
                dict(nod=nod, start=start, deg=deg, cols=cols_cb,
                     win_id=win_id, slot=slot, nwin=nwin)
            )

    NW = []
    for b in range(NBUCKETS):
        mx = max(cb_info[c * NBUCKETS + b]["nwin"] for c in range(NC))
        NW.append(((mx + 3) // 4) * 4)
    NW_total = sum(NW)
    n_sw = NW_total // 4
    wg_of_b = np.concatenate([[0], np.cumsum(NW)])[:NBUCKETS]

    in_maps = []
    metas = []
    for c in range(NC):
        col16 = np.zeros((NW_total, WIN_EDGES), np.int16)
        li = np.full((NW_total, WIN_EDGES), TRASH, np.float32)
        meta_nid = []
        meta_w = []
        meta_slot = []
        for b in range(NBUCKETS):
            info = cb_info[c * NBUCKETS + b]
            nod, start, deg = info["nod"], info["start"], info["deg"]
            cols_cb, win_id, slot, nwin = (
                info["cols"], info["win_id"], info["slot"], info["nwin"]
            )
            if len(nod) == 0:
                continue
            ordn = np.lexsort((slot, win_id))
            w_sorted = win_id[ordn]
            d_sorted = deg[ordn]
            cum = np.cumsum(d_sorted) - d_sorted
            first_idx = np.searchsorted(w_sorted, np.arange(nwin))
            off_in_win = cum - cum[first_idx][w_sorted]
            off_node = np.empty(len(nod), np.int64)
            off_node[ordn] = off_in_win
            g0 = wg_of_b[b]
            t_node = (g0 + win_id.astype(np.int64)) * WIN_EDGES + off_node
            within = np.arange(len(cols_cb)) - np.repeat(start, deg)
            targets = np.repeat(t_node, deg) + within
            col16.reshape(-1)[targets] = (cols_cb - b * BUCKET).astype(np.int16)
            li.reshape(-1)[targets] = np.repeat(slot, deg).astype(np.float32)
            meta_nid.append(nod)
            meta_w.append(g0 + win_id.astype(np.int64))
            meta_slot.append(slot.astype(np.int64))

        li_img = li.T.copy()
        idx_img = np.zeros((128, NW_total * 8), np.int16)
        for b in range(NBUCKETS):
            g0 = wg_of_b[b]
            for call0 in range(0, NW[b], CALL_CHUNKS):
                nch = min(CALL_CHUNKS, NW[b] - call0)
                lin = col16[g0 + call0 : g0 + call0 + nch].reshape(-1)
                ncols = nch * 8
                ii = np.arange(nch * WIN_EDGES)
                img16 = np.zeros((16, ncols), np.int16)
                img16[ii % 16, ii // 16] = lin
                cbase = (g0 + call0) * 8
                for k in range(8):
                    idx_img[16 * k : 16 * (k + 1), cbase : cbase + ncols] = img16

        in_maps.append(
            {"xpad": x_pad, "idx_img": idx_img, "li_img": li_img, "iota": iota}
        )
        metas.append(
            (
                np.concatenate(meta_nid) if meta_nid else np.zeros(0, np.int64),
                np.concatenate(meta_w) if meta_w else np.zeros(0, np.int64),
                np.concatenate(meta_slot) if meta_slot else np.zeros(0, np.int64),
            )
        )

    meta = {"NW": NW, "NW_total": NW_total, "n_sw": n_sw, "metas": metas}
    return in_maps, meta


def _assemble_output(results, meta):
    out = np.zeros((N_NODES, D), np.float32)
    n_sw = meta["n_sw"]
    for c in range(NC):
        st = np.asarray(results[c]["out"]).reshape(128, n_sw, D)
        nid, w, slot = meta["metas"][c]
        vals = st[(w % 4) * 32 + slot, w // 4, :]
        np.add.at(out, nid, vals)
    return out


def _build_program(NW, with_reps=False):
    NW_total = sum(NW)
    n_sw = NW_total // 4
    nc = bacc.Bacc(
        "TRN2",
        target_bir_lowering=False,
        debug=False,
        enable_asserts=False,
        num_devices=NC,
        num_swdge_queues=NSWQ,
    )
    dt = mybir.dt
    xpad_d = nc.dram_tensor("xpad", [N_NODES, EPAD], dt.float32, kind="ExternalInput")
    idx_img_d = nc.dram_tensor("idx_img", [128, NW_total * 8], dt.int16, kind="ExternalInput")
    li_img_d = nc.dram_tensor("li_img", [128, NW_total], dt.float32, kind="ExternalInput")
    iota_d = nc.dram_tensor("iota", [128, D], dt.float32, kind="ExternalInput")
    if with_reps:
        reps_d = nc.dram_tensor("reps", [1, 2], dt.int32, kind="ExternalInput")
    out_d = nc.dram_tensor("out", [128, n_sw * D], dt.float32, kind="ExternalOutput")

    with tile.TileContext(nc) as tc:
        with (
            tc.tile_pool(name="const", bufs=1) as cp,
            tc.tile_pool(name="msgs", bufs=6) as mp,
            tc.tile_pool(name="sbat", bufs=3) as sbp,
            tc.tile_pool(name="psum", bufs=4, space="PSUM") as pp,
        ):
            idx_img = cp.tile([128, NW_total * 8], dt.int16)
            li_img = cp.tile([128, NW_total], dt.float32)
            iota_t = cp.tile([128, D], dt.float32)
            nc.sync.dma_start(out=idx_img[:], in_=idx_img_d.ap())
            nc.sync.dma_start(out=li_img[:], in_=li_img_d.ap())
            nc.sync.dma_start(out=iota_t[:], in_=iota_d.ap())
            staging = cp.tile([128, n_sw * D], dt.float32)

            def body():
                w_abs = 0
                s16 = None
                ps = None
                call_i = 0
                for b in range(NBUCKETS):
                    lo = b * BUCKET
                    hi = min((b + 1) * BUCKET, N_NODES)
                    xs = xpad_d.ap()[lo:hi, :]
                    for call0 in range(0, NW[b], CALL_CHUNKS):
                        nch = min(CALL_CHUNKS, NW[b] - call0)
                        w0 = w_abs + call0
                        mg = mp.tile([128, CALL_CHUNKS, EPAD], dt.float32, tag="mg")
                        nc.gpsimd.dma_gather(
                            out_ap=mg[:, 0:nch, :],
                            in_ap=xs,
                            idxs_ap=idx_img[:, w0 * 8 : (w0 + nch) * 8],
                            num_idxs=nch * WIN_EDGES,
                            num_idxs_reg=nch * WIN_EDGES,
                            elem_size=EPAD,
                            single_packet=False,
                            queue_num=call_i % NSWQ,
                        )
                        call_i += 1
                        for k in range(nch):
                            w = w0 + k
                            if w % SBATCH == 0:
                                nb = min(SBATCH, NW_total - w)
                                s16 = sbp.tile([128, SBATCH, D], dt.float32, tag="s16")
                                li_slice = li_img[:, w : w + nb]
                                nc.vector.tensor_tensor(
                                    out=s16[:, 0:nb, :],
                                    in0=bass.AP(
                                        li_img.tensor,
                                        li_slice.offset,
                                        [li_slice.ap[0], [1, nb], [0, D]],
                                    ),
                                    in1=bass.AP(
                                        iota_t.tensor,
                                        iota_t[:].offset,
                                        [iota_t[:].ap[0], [0, nb], [1, D]],
                                    ),
                                    op=mybir.AluOpType.is_equal,
                                )
                            if w % 4 == 0:
                                ps = pp.tile([128, D], dt.float32, space="PSUM", tag="ps")
                            nc.tensor.matmul(
                                out=ps[32 * (w % 4) : 32 * (w % 4) + 32, :],
                                lhsT=s16[:, w % SBATCH, 0:32],
                                rhs=mg[:, k, 0:D],
                                start=True,
                                stop=True,
                                tile_position=(0, 32 * (w % 4)),
                            )
                            if w % 4 == 3:
                                sw = w // 4
                                nc.scalar.copy(staging[:, sw * D : (sw + 1) * D], ps[:])
                    w_abs += NW[b]
                nc.sync.dma_start(out=out_d.ap(), in_=staging[:])

            if with_reps:
                reps_t = cp.tile([1, 2], dt.int32)
                nc.sync.dma_start(out=reps_t[:], in_=reps_d.ap())
                r = nc.values_load(reps_t[0:1, 0:1])
                with tc.For_i(0, r):
                    body()
            else:
                body()

    nc.compile()
    return nc


_program_cache = {}


def kernel(x, edge_index):
    in_maps, meta = _preprocess(x, edge_index)
    key = tuple(meta["NW"])
    if key not in _program_cache:
        _program_cache[key] = _build_program(meta["NW"], with_reps=False)
    nc = _program_cache[key]
    res = run_bass_kernel_spmd(nc, in_maps, core_ids=list(range(NC)))
    return _assemble_output(res.results, meta)



# revision 6
# speedup vs baseline: 27.7930x; 1.9524x over previous
"""TRN2 Bass kernel for GNN message passing (segment-sum of gathered node rows).

out[r] = sum over edges e with row[e]==r of x[col[e]]   (N=100000, E=2000000, D=32)

Strategy (8 NeuronCores, SPMD):
  - Edges sharded by target row: core c owns rows [c*12500, (c+1)*12500).
    Outputs are disjoint per core -> no all-reduce.
  - Per core, edges split into 4 col-buckets of 25000 cols so gather indices
    fit int16 for the gpsimd dma_gather custom op (256B elems: x rows padded
    from 128B to 256B).
  - Per (core, bucket), nodes are packed into "windows" of <=31 node slots
    and exactly 128 edge slots (one PE chunk). Per chunk: one-hot S[128,32]
    built on DVE via is_equal(li, iota); one fp32 matmul S^T @ msgs
    accumulates the 32 window rows into a PSUM band (window%4); groups of 4
    windows share a [128,32] PSUM tile, copied to SBUF staging; one DMA out.
  - Host packs windows, builds the int16 index image (wrapped in 16
    partitions, replicated to all 8 Q7 stripes), and un-permutes the output.

The gather is bound by per-queue DMA descriptor processing (~8.9ns/desc on
one SWDGE queue, source- and size-independent up to 512B elems). Calls are
rotated across 4 SWDGE queues (num_swdge_queues=4, the ucode max) with a
6-deep gather pipeline, measured ~2.6ns/desc aggregate -> ~676us/iter
(3.3x over the single-queue baseline). Pad edges are minimized by
equal-width buckets + greedy degree packing.
"""

import numpy as np

import concourse.bass as bass
import concourse.bacc as bacc
import concourse.mybir as mybir
import concourse.tile as tile
from concourse.bass_utils import run_bass_kernel_spmd

N_NODES = 100000
N_EDGES = 2000000
D = 32
EPAD = 64            # padded row length (f32) -> 256B dma_gather elems
NC = 8
ROWS_PER_CORE = N_NODES // NC
BUCKET = 25000
NBUCKETS = 4
WIN_EDGES = 128
TRASH = 31
MAX_REAL = 31
CALL_CHUNKS = 32
SBATCH = 16
NSWQ = 4


def _pack_windows(degrees):
    n = len(degrees)
    order = np.argsort(degrees, kind="stable")[::-1]
    win_id = np.empty(n, np.int32)
    slot = np.empty(n, np.int32)
    degs = degrees[order]
    if n and degs[0] > WIN_EDGES:
        raise ValueError("node degree exceeds window capacity")
    hi, lo = 0, n - 1
    w = 0
    while hi <= lo:
        cap = WIN_EDGES
        s = 0
        while hi <= lo and s < MAX_REAL and degs[hi] <= cap:
            win_id[order[hi]] = w
            slot[order[hi]] = s
            cap -= degs[hi]
            s += 1
            hi += 1
        while hi <= lo and s < MAX_REAL and degs[lo] <= cap:
            win_id[order[lo]] = w
            slot[order[lo]] = s
            cap -= degs[lo]
            s += 1
            lo -= 1
        w += 1
    return win_id, slot, w


def _preprocess(x, edge_index):
    x = np.ascontiguousarray(np.asarray(x, dtype=np.float32))
    ei = np.asarray(edge_index)
    row = ei[0].astype(np.int64)
    col = ei[1].astype(np.int64)

    x_pad = np.zeros((N_NODES, EPAD), np.float32)
    x_pad[:, :D] = x
    iota = np.tile(np.arange(D, dtype=np.float32), (128, 1))

    core = (row // ROWS_PER_CORE).astype(np.int64)
    bucket = (col // BUCKET).astype(np.int64)

    key = (core * NBUCKETS + bucket) * N_NODES + row
    key_order = np.argsort(key, kind="stable")
    srow = row[key_order]
    scol = col[key_order]
    skey_cb = (core * NBUCKETS + bucket)[key_order]

    cb_info = []
    for c in range(NC):
        for b in range(NBUCKETS):
            m0 = np.searchsorted(skey_cb, c * NBUCKETS + b)
            m1 = np.searchsorted(skey_cb, c * NBUCKETS + b + 1)
            rows_cb = srow[m0:m1]
            cols_cb = scol[m0:m1]
            nod, start, deg = np.unique(rows_cb, return_index=True, return_counts=True)
            win_id, slot, nwin = _pack_windows(deg)
            cb_info.append(
                dict(nod=nod, start=start, deg=deg, cols=cols_cb,
                     win_id=win_id, slot=slot, nwin=nwin)
            )

    NW = []
    for b in range(NBUCKETS):
        mx = max(cb_info[c * NBUCKETS + b]["nwin"] for c in range(NC))
        NW.append(((mx + 3) // 4) * 4)
    NW_total = sum(NW)
    n_sw = NW_total // 4
    wg_of_b = np.concatenate([[0], np.cumsum(NW)])[:NBUCKETS]

    in_maps = []
    metas = []
    for c in range(NC):
        col16 = np.zeros((NW_total, WIN_EDGES), np.int16)
        li = np.full((NW_total, WIN_EDGES), TRASH, np.float32)
        meta_nid = []
        meta_w = []
        meta_slot = []
        for b in range(NBUCKETS):
            info = cb_info[c * NBUCKETS + b]
            nod, start, deg = info["nod"], info["start"], info["deg"]
            cols_cb, win_id, slot, nwin = (
                info["cols"], info["win_id"], info["slot"], info["nwin"]
            )
            if len(nod) == 0:
                continue
            ordn = np.lexsort((slot, win_id))
            w_sorted = win_id[ordn]
            d_sorted = deg[ordn]
            cum = np.cumsum(d_sorted) - d_sorted
            first_idx = np.searchsorted(w_sorted, np.arange(nwin))
            off_in_win = cum - cum[first_idx][w_sorted]
            off_node = np.empty(len(nod), np.int64)
            off_node[ordn] = off_in_win
            g0 = wg_of_b[b]
            t_node = (g0 + win_id.astype(np.int64)) * WIN_EDGES + off_node
            within = np.arange(len(cols_cb)) - np.repeat(start, deg)
            targets = np.repeat(t_node, deg) + within
            col16.reshape(-1)[targets] = (cols_cb - b * BUCKET).astype(np.int16)
            li.reshape(-1)[targets] = np.repeat(slot, deg).astype(np.float32)
            meta_nid.append(nod)
            meta_w.append(g0 + win_id.astype(np.int64))
            meta_slot.append(slot.astype(np.int64))

        li_img = li.T.copy()
        idx_img = np.zeros((128, NW_total * 8), np.int16)
        for b in range(NBUCKETS):
            g0 = wg_of_b[b]
            for call0 in range(0, NW[b], CALL_CHUNKS):
                nch = min(CALL_CHUNKS, NW[b] - call0)
                lin = col16[g0 + call0 : g0 + call0 + nch].reshape(-1)
                ncols = nch * 8
                ii = np.arange(nch * WIN_EDGES)
                img16 = np.zeros((16, ncols), np.int16)
                img16[ii % 16, ii // 16] = lin
                cbase = (g0 + call0) * 8
                for k in range(8):
                    idx_img[16 * k : 16 * (k + 1), cbase : cbase + ncols] = img16

        in_maps.append(
            {"xpad": x_pad, "idx_img": idx_img, "li_img": li_img, "iota": iota}
        )
        metas.append(
            (
                np.concatenate(meta_nid) if meta_nid else np.zeros(0, np.int64),
                np.concatenate(meta_w) if meta_w else np.zeros(0, np.int64),
                np.concatenate(meta_slot) if meta_slot else np.zeros(0, np.int64),
            )
        )

    meta = {"NW": NW, "NW_total": NW_total, "n_sw": n_sw, "metas": metas}
    return in_maps, meta


def _assemble_output(results, meta):
    out = np.zeros((N_NODES, D), np.float32)
    n_sw = meta["n_sw"]
    for c in range(NC):
        st = np.asarray(results[c]["out"]).reshape(128, n_sw, D)
        nid, w, slot = meta["metas"][c]
        vals = st[(w % 4) * 32 + slot, w // 4, :]
        np.add.at(out, nid, vals)
    return out


def _build_program(NW, with_reps=False):
    NW_total = sum(NW)
    n_sw = NW_total // 4
    nc = bacc.Bacc(
        "TRN2",
        target_bir_lowering=False,
        debug=False,
        enable_asserts=False,
        num_devices=NC,
        num_swdge_queues=NSWQ,
    )
    dt = mybir.dt
    xpad_d = nc.dram_tensor("xpad", [N_NODES, EPAD], dt.float32, kind="ExternalInput")
    idx_img_d = nc.dram_tensor("idx_img", [128, NW_total * 8], dt.int16, kind="ExternalInput")
    li_img_d = nc.dram_tensor("li_img", [128, NW_total], dt.float32, kind="ExternalInput")
    iota_d = nc.dram_tensor("iota", [128, D], dt.float32, kind="ExternalInput")
    if with_reps:
        reps_d = nc.dram_tensor("reps", [1, 2], dt.int32, kind="ExternalInput")
    out_d = nc.dram_tensor("out", [128, n_sw * D], dt.float32, kind="ExternalOutput")

    with tile.TileContext(nc) as tc:
        with (
            tc.tile_pool(name="const", bufs=1) as cp,
            tc.tile_pool(name="msgs", bufs=6) as mp,
            tc.tile_pool(name="sbat", bufs=3) as sbp,
            tc.tile_pool(name="psum", bufs=4, space="PSUM") as pp,
        ):
            idx_img = cp.tile([128, NW_total * 8], dt.int16)
            li_img = cp.tile([128, NW_total], dt.float32)
            iota_t = cp.tile([128, D], dt.float32)
            nc.sync.dma_start(out=idx_img[:], in_=idx_img_d.ap())
            nc.sync.dma_start(out=li_img[:], in_=li_img_d.ap())
            nc.sync.dma_start(out=iota_t[:], in_=iota_d.ap())
            staging = cp.tile([128, n_sw * D], dt.float32)

            def body():
                w_abs = 0
                s16 = None
                ps = None
                call_i = 0
                for b in range(NBUCKETS):
                    lo = b * BUCKET
                    hi = min((b + 1) * BUCKET, N_NODES)
                    xs = xpad_d.ap()[lo:hi, :]
                    for call0 in range(0, NW[b], CALL_CHUNKS):
                        nch = min(CALL_CHUNKS, NW[b] - call0)
                        w0 = w_abs + call0
                        mg = mp.tile([128, CALL_CHUNKS, EPAD], dt.float32, tag="mg")
                        nc.gpsimd.dma_gather(
                            out_ap=mg[:, 0:nch, :],
                            in_ap=xs,
                            idxs_ap=idx_img[:, w0 * 8 : (w0 + nch) * 8],
                            num_idxs=nch * WIN_EDGES,
                            num_idxs_reg=nch * WIN_EDGES,
                            elem_size=EPAD,
                            single_packet=False,
                            queue_num=call_i % NSWQ,
                        )
                        call_i += 1
                        for k in range(nch):
                            w = w0 + k
                            if w % SBATCH == 0:
                                nb = min(SBATCH, NW_total - w)
                                s16 = sbp.tile([128, SBATCH, D], dt.float32, tag="s16")
                                li_slice = li_img[:, w : w + nb]
                                nc.vector.tensor_tensor(
                                    out=s16[:, 0:nb, :],
                                    in0=bass.AP(
                                        li_img.tensor,
                                        li_slice.offset,
                                        [li_slice.ap[0], [1, nb], [0, D]],
                                    ),
                                    in1=bass.AP(
                                        iota_t.tensor,
                                        iota_t[:].offset,
                                        [iota_t[:].ap[0], [0, nb], [1, D]],
                                    ),
                                    op=mybir.AluOpType.is_equal,
                                )
                            if w % 4 == 0:
                                ps = pp.tile([128, D], dt.float32, space="PSUM", tag="ps")
                            nc.tensor.matmul(
                                out=ps[32 * (w % 4) : 32 * (w % 4) + 32, :],
                                lhsT=s16[:, w % SBATCH, 0:32],
                                rhs=mg[:, k, 0:D],
                                start=True,
                                stop=True,
                                tile_position=(0, 32 * (w % 4)),
                            )
                            if w % 4 == 3:
                                sw = w // 4
                                nc.scalar.copy(staging[:, sw * D : (sw + 1) * D], ps[:])
                    w_abs += NW[b]
                nc.sync.dma_start(out=out_d.ap(), in_=staging[:])

            if with_reps:
                reps_t = cp.tile([1, 2], dt.int32)
                nc.sync.dma_start(out=reps_t[:], in_=reps_d.ap())
                r = nc.values_load(reps_t[0:1, 0:1])
                with tc.For_i(0, r):
                    body()
            else:
                body()

    nc.compile()
    return nc


_program_cache = {}


def kernel(x, edge_index):
    in_maps, meta = _preprocess(x, edge_index)
    key = tuple(meta["NW"])
    if key not in _program_cache:
        _program_cache[key] = _build_program(meta["NW"], with_reps=False)
    nc = _program_cache[key]
    res = run_bass_kernel_spmd(nc, in_maps, core_ids=list(range(NC)))
    return _assemble_output(res.results, meta)



# revision 8
# speedup vs baseline: 51.2895x; 1.8454x over previous
"""TRN2 Bass kernel for GNN message passing (segment-sum of gathered node rows).

out[r] = sum over edges e with row[e]==r of x[col[e]]   (N=100000, E=2000000, D=32)

Strategy (8 NeuronCores, SPMD):
  - Edges sharded by target row: core c owns rows [c*12500, (c+1)*12500).
    Outputs are disjoint per core -> no all-reduce.
  - Per core, edges split into 4 col-buckets of 25000 cols so gather indices
    fit int16 for the gpsimd dma_gather custom op (256B elems: x rows padded
    from 128B to 256B).
  - Per (core, bucket), nodes are packed into "windows" of <=63 node slots
    and exactly 128 edge slots (one PE chunk). Per chunk: one-hot S[128,64]
    built on DVE via is_equal(li, iota64); one fp32 matmul S^T @ msgs
    accumulates the 64 window rows into a PSUM band (window%2); pairs of
    windows share a [128,32] PSUM tile, copied to chunked SBUF staging
    tiles DMA'd out every 64 pairs.
  - Host packs windows, builds the int16 index image (wrapped in 16
    partitions, replicated to all 8 Q7 stripes), and un-permutes the output.

The gather is bound by per-queue DMA descriptor processing (~8.9ns/desc on
one SWDGE queue, source- and size-independent up to 512B elems). Calls are
rotated across 4 SWDGE queues (num_swdge_queues=4, the ucode max) with a
6-deep gather pipeline, measured ~2.6ns/desc aggregate -> ~676us/iter
(3.3x over the single-queue baseline). Pad edges are minimized by
equal-width buckets + greedy degree packing.
"""

import numpy as np

import concourse.bass as bass
import concourse.bacc as bacc
import concourse.mybir as mybir
import concourse.tile as tile
from concourse.bass_utils import run_bass_kernel_spmd

N_NODES = 100000
N_EDGES = 2000000
D = 32
EPAD = 64            # padded row length (f32) -> 256B dma_gather elems
NC = 8
ROWS_PER_CORE = N_NODES // NC
BUCKET = 25000
NBUCKETS = 4
WIN_EDGES = 128
SLOTW = 64
TRASH = 63
MAX_REAL = 63
CALL_CHUNKS = 32
SBATCH = 8
STG = 64
NSWQ = 4


def _pack_windows(degrees):
    n = len(degrees)
    order = np.argsort(degrees, kind="stable")[::-1]
    win_id = np.empty(n, np.int32)
    slot = np.empty(n, np.int32)
    degs = degrees[order]
    if n and degs[0] > WIN_EDGES:
        raise ValueError("node degree exceeds window capacity")
    hi, lo = 0, n - 1
    w = 0
    while hi <= lo:
        cap = WIN_EDGES
        s = 0
        while hi <= lo and s < MAX_REAL and degs[hi] <= cap:
            win_id[order[hi]] = w
            slot[order[hi]] = s
            cap -= degs[hi]
            s += 1
            hi += 1
        while hi <= lo and s < MAX_REAL and degs[lo] <= cap:
            win_id[order[lo]] = w
            slot[order[lo]] = s
            cap -= degs[lo]
            s += 1
            lo -= 1
        w += 1
    return win_id, slot, w


def _preprocess(x, edge_index):
    x = np.ascontiguousarray(np.asarray(x, dtype=np.float32))
    ei = np.asarray(edge_index)
    row = ei[0].astype(np.int64)
    col = ei[1].astype(np.int64)

    x_pad = np.zeros((N_NODES, EPAD), np.float32)
    x_pad[:, :D] = x
    iota = np.tile(np.arange(SLOTW, dtype=np.float32), (128, 1))

    core = (row // ROWS_PER_CORE).astype(np.int64)
    bucket = (col // BUCKET).astype(np.int64)

    key = (core * NBUCKETS + bucket) * N_NODES + row
    key_order = np.argsort(key, kind="stable")
    srow = row[key_order]
    scol = col[key_order]
    skey_cb = (core * NBUCKETS + bucket)[key_order]

    cb_info = []
    for c in range(NC):
        for b in range(NBUCKETS):
            m0 = np.searchsorted(skey_cb, c * NBUCKETS + b)
            m1 = np.searchsorted(skey_cb, c * NBUCKETS + b + 1)
            rows_cb = srow[m0:m1]
            cols_cb = scol[m0:m1]
            nod, start, deg = np.unique(rows_cb, return_index=True, return_counts=True)
            win_id, slot, nwin = _pack_windows(deg)
            cb_info.append(
                dict(nod=nod, start=start, deg=deg, cols=cols_cb,
                     win_id=win_id, slot=slot, nwin=nwin)
            )

    NW = []
    for b in range(NBUCKETS):
        mx = max(cb_info[c * NBUCKETS + b]["nwin"] for c in range(NC))
        NW.append(((mx + 1) // 2) * 2)
    NW_total = sum(NW)
    n_sw = NW_total // 2
    wg_of_b = np.concatenate([[0], np.cumsum(NW)])[:NBUCKETS]

    in_maps = []
    metas = []
    for c in range(NC):
        col16 = np.zeros((NW_total, WIN_EDGES), np.int16)
        li = np.full((NW_total, WIN_EDGES), TRASH, np.float32)
        meta_nid = []
        meta_w = []
        meta_slot = []
        for b in range(NBUCKETS):
            info = cb_info[c * NBUCKETS + b]
            nod, start, deg = info["nod"], info["start"], info["deg"]
            cols_cb, win_id, slot, nwin = (
                info["cols"], info["win_id"], info["slot"], info["nwin"]
            )
            if len(nod) == 0:
                continue
            ordn = np.lexsort((slot, win_id))
            w_sorted = win_id[ordn]
            d_sorted = deg[ordn]
            cum = np.cumsum(d_sorted) - d_sorted
            first_idx = np.searchsorted(w_sorted, np.arange(nwin))
            off_in_win = cum - cum[first_idx][w_sorted]
            off_node = np.empty(len(nod), np.int64)
            off_node[ordn] = off_in_win
            g0 = wg_of_b[b]
            t_node = (g0 + win_id.astype(np.int64)) * WIN_EDGES + off_node
            within = np.arange(len(cols_cb)) - np.repeat(start, deg)
            targets = np.repeat(t_node, deg) + within
            col16.reshape(-1)[targets] = (cols_cb - b * BUCKET).astype(np.int16)
            li.reshape(-1)[targets] = np.repeat(slot, deg).astype(np.float32)
            meta_nid.append(nod)
            meta_w.append(g0 + win_id.astype(np.int64))
            meta_slot.append(slot.astype(np.int64))

        li_img = li.T.copy()
        idx_img = np.zeros((128, NW_total * 8), np.int16)
        for b in range(NBUCKETS):
            g0 = wg_of_b[b]
            for call0 in range(0, NW[b], CALL_CHUNKS):
                nch = min(CALL_CHUNKS, NW[b] - call0)
                lin = col16[g0 + call0 : g0 + call0 + nch].reshape(-1)
                ncols = nch * 8
                ii = np.arange(nch * WIN_EDGES)
                img16 = np.zeros((16, ncols), np.int16)
                img16[ii % 16, ii // 16] = lin
                cbase = (g0 + call0) * 8
                for k in range(8):
                    idx_img[16 * k : 16 * (k + 1), cbase : cbase + ncols] = img16

        in_maps.append(
            {"xpad": x_pad, "idx_img": idx_img, "li_img": li_img, "iota": iota}
        )
        metas.append(
            (
                np.concatenate(meta_nid) if meta_nid else np.zeros(0, np.int64),
                np.concatenate(meta_w) if meta_w else np.zeros(0, np.int64),
                np.concatenate(meta_slot) if meta_slot else np.zeros(0, np.int64),
            )
        )

    meta = {"NW": NW, "NW_total": NW_total, "n_sw": n_sw, "metas": metas}
    return in_maps, meta


def _assemble_output(results, meta):
    out = np.zeros((N_NODES, D), np.float32)
    n_sw = meta["n_sw"]
    for c in range(NC):
        st = np.asarray(results[c]["out"]).reshape(128, n_sw, D)
        nid, w, slot = meta["metas"][c]
        vals = st[(w % 2) * 64 + slot, w // 2, :]
        np.add.at(out, nid, vals)
    return out


def _build_program(NW, with_reps=False):
    NW_total = sum(NW)
    n_sw = NW_total // 2
    nc = bacc.Bacc(
        "TRN2",
        target_bir_lowering=False,
        debug=False,
        enable_asserts=False,
        num_devices=NC,
        num_swdge_queues=NSWQ,
    )
    dt = mybir.dt
    xpad_d = nc.dram_tensor("xpad", [N_NODES, EPAD], dt.float32, kind="ExternalInput")
    idx_img_d = nc.dram_tensor("idx_img", [128, NW_total * 8], dt.int16, kind="ExternalInput")
    li_img_d = nc.dram_tensor("li_img", [128, NW_total], dt.float32, kind="ExternalInput")
    iota_d = nc.dram_tensor("iota", [128, SLOTW], dt.float32, kind="ExternalInput")
    if with_reps:
        reps_d = nc.dram_tensor("reps", [1, 2], dt.int32, kind="ExternalInput")
    out_d = nc.dram_tensor("out", [128, n_sw * D], dt.float32, kind="ExternalOutput")

    with tile.TileContext(nc) as tc:
        with (
            tc.tile_pool(name="const", bufs=1) as cp,
            tc.tile_pool(name="msgs", bufs=8) as mp,
            tc.tile_pool(name="sbat", bufs=3) as sbp,
            tc.tile_pool(name="stg", bufs=2) as stgp,
            tc.tile_pool(name="psum", bufs=4, space="PSUM") as pp,
        ):
            idx_img = cp.tile([128, NW_total * 8], dt.int16)
            li_img = cp.tile([128, NW_total], dt.float32)
            iota_t = cp.tile([128, SLOTW], dt.float32)
            nc.sync.dma_start(out=idx_img[:], in_=idx_img_d.ap())
            nc.sync.dma_start(out=li_img[:], in_=li_img_d.ap())
            nc.sync.dma_start(out=iota_t[:], in_=iota_d.ap())

            def body():
                w_abs = 0
                s16 = None
                ps = None
                call_i = 0
                state = {"stg": None, "base": 0}
                for b in range(NBUCKETS):
                    lo = b * BUCKET
                    hi = min((b + 1) * BUCKET, N_NODES)
                    xs = xpad_d.ap()[lo:hi, :]
                    for call0 in range(0, NW[b], CALL_CHUNKS):
                        nch = min(CALL_CHUNKS, NW[b] - call0)
                        w0 = w_abs + call0
                        mg = mp.tile([128, CALL_CHUNKS, EPAD], dt.float32, tag="mg")
                        nc.gpsimd.dma_gather(
                            out_ap=mg[:, 0:nch, :],
                            in_ap=xs,
                            idxs_ap=idx_img[:, w0 * 8 : (w0 + nch) * 8],
                            num_idxs=nch * WIN_EDGES,
                            num_idxs_reg=nch * WIN_EDGES,
                            elem_size=EPAD,
                            single_packet=False,
                            queue_num=call_i % NSWQ,
                        )
                        call_i += 1
                        for k in range(nch):
                            w = w0 + k
                            if w % SBATCH == 0:
                                nb = min(SBATCH, NW_total - w)
                                s16 = sbp.tile([128, SBATCH, SLOTW], dt.float32, tag="s16")
                                li_slice = li_img[:, w : w + nb]
                                nc.vector.tensor_tensor(
                                    out=s16[:, 0:nb, :],
                                    in0=bass.AP(
                                        li_img.tensor,
                                        li_slice.offset,
                                        [li_slice.ap[0], [1, nb], [0, SLOTW]],
                                    ),
                                    in1=bass.AP(
                                        iota_t.tensor,
                                        iota_t[:].offset,
                                        [iota_t[:].ap[0], [0, nb], [1, SLOTW]],
                                    ),
                                    op=mybir.AluOpType.is_equal,
                                )
                            if w % 2 == 0:
                                ps = pp.tile([128, D], dt.float32, space="PSUM", tag="ps")
                            nc.tensor.matmul(
                                out=ps[64 * (w % 2) : 64 * (w % 2) + SLOTW, :],
                                lhsT=s16[:, w % SBATCH, 0:SLOTW],
                                rhs=mg[:, k, 0:D],
                                start=True,
                                stop=True,
                                tile_position=(0, 64 * (w % 2)),
                            )
                            if w % 2 == 1:
                                sw = w // 2
                                if sw % STG == 0:
                                    if state.get("stg") is not None:
                                        nc.sync.dma_start(
                                            out=out_d.ap()[:, state["base"] * D : (state["base"] + STG) * D],
                                            in_=state["stg"][:],
                                        )
                                    state["stg"] = stgp.tile([128, STG, D], dt.float32, tag="stg", name="stg")
                                    state["base"] = sw
                                nc.scalar.copy(state["stg"][:, sw - state["base"], :], ps[:])
                    w_abs += NW[b]
                left = n_sw - state["base"]
                nc.sync.dma_start(
                    out=out_d.ap()[:, state["base"] * D : (state["base"] + left) * D],
                    in_=state["stg"][:, 0:left, :],
                )

            if with_reps:
                reps_t = cp.tile([1, 2], dt.int32)
                nc.sync.dma_start(out=reps_t[:], in_=reps_d.ap())
                r = nc.values_load(reps_t[0:1, 0:1])
                with tc.For_i(0, r):
                    body()
            else:
                body()

    nc.compile()
    return nc


_program_cache = {}


def kernel(x, edge_index):
    in_maps, meta = _preprocess(x, edge_index)
    key = tuple(meta["NW"])
    if key not in _program_cache:
        _program_cache[key] = _build_program(meta["NW"], with_reps=False)
    nc = _program_cache[key]
    res = run_bass_kernel_spmd(nc, in_maps, core_ids=list(range(NC)))
    return _assemble_output(res.results, meta)

